# revision 6
# baseline (speedup 1.0000x reference)
"""GCN block (2-layer GCNConv + ReLU) on 8 Trainium2 NeuronCores.

Strategy (1D node partitioning, v3 — dma_gather based):
  - Core c owns target nodes [c*6250, (c+1)*6250) and all edges whose target
    lands there. Aggregation is reordered before the weight matmul:
    A_norm @ (x W) == (A_norm @ x) W.
  - Node tables live in DRAM as [25600, 128] f16 row pairs (96 features +
    pad to the 256B row the dma_gather ucode requires), split into a lo and
    a hi tensor so the int16 gather indices stay in range. The padded-global
    layout is piece-major: within each half, [core][local row], with
    per-core rows padded to npad/2 = 3200 (25 blocks).
  - Per-edge source rows are fetched with the custom SWDGE dma_gather
    instruction, 1024 indices per instruction (the ucode's limit),
    round-robin across 4 SWDGE queues (4 queues ≈ 6.6x the gather
    throughput of one).
  - Each core's edge stream is split by source half (lo/hi) and sorted by
    target block (128 targets). For each 128-edge chunk-segment, a selection
    matrix S[e, m] = w[e] * (localtgt[e] == m) is built on the vector engine
    in 32-seg batches; w = 0 masks positions outside the segment's span and
    stream padding. The tensor engine accumulates
    aggT[96, 128] = sum_seg chunk[:, :96].T @ S in PSUM — feature-major, so
    the dense 96x96 weight matmuls need no transposes.
  - Self-loops are folded in as ordinary edges with weight dinv^2.
  - Layer 1 per block: aggT -> W1 -> relu+b1 -> W2 -> transpose -> t2_own
    rows. The halo exchange is TWO AllGathers (one per half), each issued
    as soon as its 25 blocks are done, so the first one's transfer overlaps
    the second half of layer-1 compute.
  - Layer 2 per block: aggT -> relu+b2 -> feature-major output [96, npad];
    the host transposes back.
"""

import os
import sys

for _p in ("/opt/trn_rl_repo", "/root/.axon_site/_ro/trn_rl_repo"):
    if os.path.isdir(_p) and _p not in sys.path:
        sys.path.insert(0, _p)

import numpy as np

import concourse.bacc as bacc
import concourse.bass as bass
import concourse.mybir as mybir
import concourse.tile as tile
from concourse import bass_utils

F16 = mybir.dt.float16
F32 = mybir.dt.float32
I16 = mybir.dt.int16

P = 128          # partitions / edges per chunk / nodes per target block
D = 96           # feature dim
NCORES = 8
G = 64           # chunks per gather slab (one SBUF tile)
GI = 8           # chunks per dma_gather instruction (GI*128 idxs each)
NQ = 4           # SWDGE queues for gather descriptor generation
SBATCH = 32      # segments per S-matrix build batch


def _preprocess(row, col, ew, N):
    """Per-core edge streams for the gather/S-matmul schedule.

    Returns metadata shared across cores (schedule) plus per-core arrays.
    """
    npc = N // NCORES                 # 6250
    nblk = (npc + P - 1) // P         # 49
    nblk += nblk % 2                  # 50 (even, for the 2-piece halo)
    npad = nblk * P                   # 6400
    pr = npad // 2                    # rows per piece per core (3200)
    half = NCORES * pr                # rows per table half (25600)

    deg = np.bincount(col, weights=ew, minlength=N) + 1.0
    dinv = (1.0 / np.sqrt(deg)).astype(np.float32)
    norm = (dinv[row] * ew * dinv[col]).astype(np.float32)
    selfn = (dinv * dinv).astype(np.float32)

    def gmap(nodes):
        c = nodes // npc
        l = nodes - c * npc
        piece = (l >= pr).astype(np.int64)
        return piece * half + c * pr + (l - piece * pr)

    g_all = gmap(row)

    per_core = []
    counts = np.zeros((NCORES, 2, nblk), np.int64)
    tsel = np.arange(npc, dtype=np.int64)
    for c in range(NCORES):
        m = (col >= c * npc) & (col < (c + 1) * npc)
        src_g = np.concatenate([g_all[m], gmap(c * npc + tsel)])
        tgt = np.concatenate([col[m] - c * npc, tsel])
        w = np.concatenate([norm[m], selfn[c * npc + tsel]]).astype(np.float32)
        st = src_g >= half
        blk = tgt // P
        data = []
        for t in (0, 1):
            mm = st == (t == 1)
            o = np.argsort(blk[mm], kind="stable")
            data.append((
                (src_g[mm][o] - t * half).astype(np.int64),
                (tgt[mm][o] - blk[mm][o] * P).astype(np.int64),
                w[mm][o],
            ))
            counts[c, t] = np.bincount(blk[mm], minlength=nblk)
        per_core.append(data)

    nb = counts.max(axis=0)           # [2, nblk] padded per-block counts
    L = nb.sum(axis=1)
    Ct = [(int(L[t]) + P - 1) // P for t in (0, 1)]
    for t in (0, 1):
        nb[t, -1] += Ct[t] * P - L[t]
    starts = np.zeros((2, nblk + 1), np.int64)
    starts[:, 1:] = np.cumsum(nb, axis=1)

    # segment schedule in consumption order (per block: lo spans, hi spans)
    segs = []  # (t, cch, b, lo_, hi_)
    for b in range(nblk):
        for t in (0, 1):
            s, e = int(starts[t, b]), int(starts[t, b + 1])
            if s == e:
                continue
            for cch in range(s // P, (e - 1) // P + 1):
                segs.append((t, cch, b,
                             max(s, cch * P) - cch * P,
                             min(e, (cch + 1) * P) - cch * P))
    nseg = len(segs)

    idxw = [np.zeros((NCORES, P, Ct[t] * 8), np.int16) for t in (0, 1)]
    colg = np.zeros((NCORES, P, nseg), np.float16)
    wg = np.zeros((NCORES, P, nseg), np.float16)
    for c in range(NCORES):
        for t in (0, 1):
            sg, tg, wwv = per_core[c][t]
            Lp = Ct[t] * P
            si = np.zeros(Lp, np.int64)
            sc = np.zeros(Lp, np.int64)
            sw = np.zeros(Lp, np.float32)
            e0 = 0
            for b in range(nblk):
                n = int(counts[c, t, b])
                s = int(starts[t, b])
                si[s:s + n] = sg[e0:e0 + n]
                sc[s:s + n] = tg[e0:e0 + n]
                sw[s:s + n] = wwv[e0:e0 + n]
                e0 += n
            idxw[t][c] = np.tile(
                si.astype(np.int16).reshape(-1, 16).T, (8, 1))
            scP = sc.reshape(Ct[t], P).T
            swP = sw.reshape(Ct[t], P).T
            for k, (tt, cch, _b, lo_, hi_) in enumerate(segs):
                if tt == t:
                    colg[c, lo_:hi_, k] = scP[lo_:hi_, cch]
                    wg[c, lo_:hi_, k] = swP[lo_:hi_, cch]

    return {
        "npc": npc, "nblk": nblk, "npad": npad, "half": half, "pr": pr,
        "Ct": tuple(Ct), "segs": tuple(segs), "nseg": nseg,
        "idxlo": idxw[0], "idxhi": idxw[1], "colg": colg, "wg": wg,
    }


def _build_program(N, npad, half, nblk, Ct, segs, repeat=1, mode="full"):
    nseg = len(segs)
    pr = npad // 2
    prb = pr // P                     # blocks per piece
    nc = bacc.Bacc("TRN2", target_bir_lowering=False, debug=False,
                   enable_asserts=False, num_devices=NCORES,
                   num_swdge_queues=NQ)

    x_d = [nc.dram_tensor(f"x{t}", [half, P], F16,
                          kind="ExternalInput").ap() for t in (0, 1)]
    idx_d = [nc.dram_tensor(f"idx{t}", [P, Ct[t] * 8], I16,
                            kind="ExternalInput").ap() for t in (0, 1)]
    colg_d = nc.dram_tensor("colg", [P, nseg], F16, kind="ExternalInput").ap()
    wg_d = nc.dram_tensor("wg", [P, nseg], F16, kind="ExternalInput").ap()
    iota_d = nc.dram_tensor("iota", [P, P], F16, kind="ExternalInput").ap()
    ident_d = nc.dram_tensor("ident", [D, D], F16, kind="ExternalInput").ap()
    w1_d = nc.dram_tensor("w1", [D, D], F16, kind="ExternalInput").ap()
    w2_d = nc.dram_tensor("w2", [D, D], F16, kind="ExternalInput").ap()
    b1_d = nc.dram_tensor("b1", [D, 1], F32, kind="ExternalInput").ap()
    b2_d = nc.dram_tensor("b2", [D, 1], F32, kind="ExternalInput").ap()
    out_d = nc.dram_tensor("out", [D, npad], F32, kind="ExternalOutput").ap()

    nslab = [(Ct[t] + G - 1) // G for t in (0, 1)]

    with tile.TileContext(nc) as tc:
        with (
            tc.tile_pool(name="const", bufs=1) as const_pool,
            tc.tile_pool(name="meta", bufs=1) as meta_pool,
            tc.tile_pool(name="glo", bufs=4) as glo_pool,
            tc.tile_pool(name="ghi", bufs=4) as ghi_pool,
            tc.tile_pool(name="sbig", bufs=3) as sbig_pool,
            tc.tile_pool(name="work", bufs=2) as w_pool,
            tc.tile_pool(name="pagg", bufs=3, space="PSUM") as pagg_pool,
            tc.tile_pool(name="pmm", bufs=1, space="PSUM") as pmm_pool,
            tc.tile_pool(name="dram", bufs=1, space="DRAM") as dram_pool,
        ):
            iota_sb = const_pool.tile([P, P], F16, tag="iota")
            ident_sb = const_pool.tile([D, D], F16, tag="ident")
            w1_sb = const_pool.tile([D, D], F16, tag="w1")
            w2_sb = const_pool.tile([D, D], F16, tag="w2")
            b1_sb = const_pool.tile([D, 1], F32, tag="b1")
            b2_sb = const_pool.tile([D, 1], F32, tag="b2")
            for sb, dr in ((iota_sb, iota_d), (ident_sb, ident_d),
                           (w1_sb, w1_d), (w2_sb, w2_d),
                           (b1_sb, b1_d), (b2_sb, b2_d)):
                nc.sync.dma_start(sb[:], dr[:])

            idx_sb = [meta_pool.tile([P, Ct[t] * 8], I16, tag=f"idx{t}",
                                     name=f"idx{t}") for t in (0, 1)]
            colg_sb = meta_pool.tile([P, nseg], F16, tag="colg")
            wg_sb = meta_pool.tile([P, nseg], F16, tag="wg")
            for t in (0, 1):
                nc.sync.dma_start(idx_sb[t][:], idx_d[t][:])
            nc.sync.dma_start(colg_sb[:], colg_d[:])
            nc.sync.dma_start(wg_sb[:], wg_d[:])

            gdummy = None
            if mode in ("nog", "sonly"):
                gdummy = const_pool.tile([P, G * P], F16, tag="gdummy")
                nc.vector.memset(gdummy[:], 0.0)

            t2own = dram_pool.tile([npad, P], F16, tag="t2own")
            t2piece = [
                [dram_pool.tile([half, P], F16, tag=f"t2p{t}r{r}",
                                addr_space="Shared", name=f"t2p{t}r{r}")
                 for t in (0, 1)]
                for r in range(repeat)
            ]

            g_pools = (glo_pool, ghi_pool)
            qctr = [0]

            def issue_gather(tabs, t, k, gtiles):
                if mode in ("nog", "sonly"):
                    gtiles[(t, k)] = gdummy
                    return
                rem = min(G, Ct[t] - k * G)
                gbuf = g_pools[t].tile([P, G * P], F16, tag=f"g{t}",
                                       name=f"g{t}")
                for j in range(0, rem, GI):
                    gi = min(GI, rem - j)
                    c0 = k * G + j
                    nc.gpsimd.dma_gather(
                        gbuf[:, j * P:(j + gi) * P].rearrange(
                            "p (c e) -> p c e", e=P),
                        tabs[t],
                        idx_sb[t][:, c0 * 8: (c0 + gi) * 8],
                        gi * P,
                        gi * P,
                        P,
                        elem_step=P,
                        queue_num=qctr[0] % NQ,
                    )
                    qctr[0] += 1
                gtiles[(t, k)] = gbuf

            def post_block(layer, b, psum):
                if layer == 0:
                    h_in = w_pool.tile([D, P], F16, tag="h_in")
                    nc.scalar.activation(h_in[:], psum[:],
                                         mybir.ActivationFunctionType.Copy)
                    pz = pmm_pool.tile([D, P], F32, tag="pz")
                    nc.tensor.matmul(out=pz[:], lhsT=w1_sb[:], rhs=h_in[:],
                                     start=True, stop=True)
                    h1 = w_pool.tile([D, P], F16, tag="h1")
                    nc.scalar.activation(h1[:], pz[:],
                                         mybir.ActivationFunctionType.Relu,
                                         bias=b1_sb[:], scale=1.0)
                    pt2 = pmm_pool.tile([D, P], F32, tag="pt2")
                    nc.tensor.matmul(out=pt2[:], lhsT=w2_sb[:], rhs=h1[:],
                                     start=True, stop=True)
                    t2c = w_pool.tile([D, P], F16, tag="t2c")
                    nc.vector.tensor_copy(t2c[:], pt2[:])
                    ptr = pmm_pool.tile([P, D], F16, tag="ptr")
                    nc.tensor.transpose(ptr[:], t2c[:], ident_sb[:])
                    t2n = w_pool.tile([P, P], F16, tag="t2n")
                    nc.vector.memset(t2n[:, D:], 0.0)
                    nc.vector.tensor_copy(t2n[:, :D], ptr[:])
                    nc.sync.dma_start(t2own[b * P:(b + 1) * P, :], t2n[:])
                else:
                    o_sb = w_pool.tile([D, P], F32, tag="o_sb")
                    nc.scalar.activation(o_sb[:], psum[:],
                                         mybir.ActivationFunctionType.Relu,
                                         bias=b2_sb[:], scale=1.0)
                    nc.sync.dma_start(out_d[:, b * P:(b + 1) * P], o_sb[:])

            for rep_i, layer in enumerate([0, 1] * repeat):
                t2full = t2piece[rep_i // 2]
                if layer == 0:
                    tabs = (x_d[0], x_d[1])
                else:
                    tabs = (t2full[0][:], t2full[1][:])

                gtiles = {}
                touched = [-1, -1]
                for t in (0, 1):
                    for k in range(min(3, nslab[t])):
                        issue_gather(tabs, t, k, gtiles)

                psum = None
                cur_b = -1
                sb_tile = None
                for si, (t, cch, b, lo_, hi_) in enumerate(segs):
                    k = cch // G
                    if k > touched[t]:
                        touched[t] = k
                        if k + 3 < nslab[t]:
                            issue_gather(tabs, t, k + 3, gtiles)
                    if si % SBATCH == 0:
                        n2 = min(SBATCH, nseg - si)
                        sb_tile = sbig_pool.tile([P, SBATCH * P], F16,
                                                 tag="sbig", name="sbig")
                        vw = sb_tile[:, :n2 * P].rearrange(
                            "p (g m) -> p g m", m=P)
                        ia = iota_sb[:]
                        ca = colg_sb[:, si:si + n2]
                        wa = wg_sb[:, si:si + n2]
                        ap_i = bass.AP(ia.tensor, ia.offset,
                                       [list(ia.ap[0]), [0, n2], [1, P]])
                        ap_c = bass.AP(ca.tensor, ca.offset,
                                       [list(ca.ap[0]), list(ca.ap[1]),
                                        [0, P]])
                        ap_w = bass.AP(wa.tensor, wa.offset,
                                       [list(wa.ap[0]), list(wa.ap[1]),
                                        [0, P]])
                        nc.vector.tensor_tensor(
                            out=vw, in0=ap_i, in1=ap_c,
                            op=mybir.AluOpType.is_equal)
                        nc.vector.tensor_tensor(
                            out=vw, in0=vw, in1=ap_w,
                            op=mybir.AluOpType.mult)

                    if mode in ("gonly", "sonly"):
                        continue
                    if b != cur_b:
                        psum = pagg_pool.tile([D, P], F32, tag="pagg",
                                              name="pagg")
                        cur_b = b
                    gbuf = gtiles[(t, k)]
                    c_local = cch - k * G
                    first = (si == 0) or segs[si - 1][2] != b
                    last = (si == nseg - 1) or segs[si + 1][2] != b
                    nc.tensor.matmul(
                        out=psum[:],
                        lhsT=gbuf[:, c_local * P:c_local * P + D],
                        rhs=sb_tile[:, (si % SBATCH) * P:
                                    (si % SBATCH + 1) * P],
                        start=first, stop=last,
                    )
                    if last:
                        post_block(layer, b, psum)
                        if layer == 0 and mode == "full" and \
                                (b == prb - 1 or b == nblk - 1):
                            piece = 0 if b == prb - 1 else 1
                            nc.gpsimd.collective_compute(
                                "AllGather",
                                mybir.AluOpType.bypass,
                                replica_groups=[list(range(NCORES))],
                                ins=[t2own[piece * pr:(piece + 1) * pr, :]],
                                outs=[t2full[piece][:]],
                            )

    nc.compile()
    return nc


_CACHE = {}


def _get_program(N, npad, half, nblk, Ct, segs, repeat=1, mode="full"):
    key = (N, npad, half, nblk, Ct, segs, repeat, mode)
    if key not in _CACHE:
        _CACHE[key] = _build_program(N, npad, half, nblk, Ct, list(segs),
                                     repeat=repeat, mode=mode)
    return _CACHE[key]


def _make_inputs(x, W1, b1, W2, b2, pre):
    npc, npad, pr, half = pre["npc"], pre["npad"], pre["pr"], pre["half"]
    xs = np.asarray(x, np.float32).astype(np.float16)
    xt = [np.zeros((half, P), np.float16) for _ in (0, 1)]
    for c in range(NCORES):
        rows = xs[c * npc:(c + 1) * npc]
        n0 = min(pr, rows.shape[0])
        xt[0][c * pr:c * pr + n0, :D] = rows[:n0]
        xt[1][c * pr:c * pr + rows.shape[0] - n0, :D] = rows[n0:]
    common = {
        "x0": xt[0],
        "x1": xt[1],
        "iota": np.tile(np.arange(P, dtype=np.float16), (P, 1)),
        "ident": np.eye(D, dtype=np.float16),
        "w1": np.asarray(W1, np.float32).astype(np.float16),
        "w2": np.asarray(W2, np.float32).astype(np.float16),
        "b1": np.asarray(b1, np.float32).reshape(D, 1),
        "b2": np.asarray(b2, np.float32).reshape(D, 1),
    }
    in_maps = []
    for c in range(NCORES):
        m = dict(common)
        m["idx0"] = pre["idxlo"][c]
        m["idx1"] = pre["idxhi"][c]
        m["colg"] = pre["colg"][c]
        m["wg"] = pre["wg"][c]
        in_maps.append(m)
    return in_maps


def kernel(x, edge_index, edge_weight, batch, W1, b1, W2, b2, **_unused):
    x = np.asarray(x, dtype=np.float32)
    edge_index = np.asarray(edge_index)
    ew = np.asarray(edge_weight, dtype=np.float32)
    N = x.shape[0]
    row = np.asarray(edge_index[0], dtype=np.int64)
    col = np.asarray(edge_index[1], dtype=np.int64)

    pre = _preprocess(row, col, ew, N)
    nc = _get_program(N, pre["npad"], pre["half"], pre["nblk"],
                      pre["Ct"], pre["segs"])
    in_maps = _make_inputs(x, W1, b1, W2, b2, pre)

    res = bass_utils.run_bass_kernel_spmd(nc, in_maps,
                                          core_ids=list(range(NCORES)))
    npc = pre["npc"]
    out = np.concatenate(
        [res.results[c]["out"][:, :npc].T for c in range(NCORES)], axis=0)
    return out.astype(np.float32)
